# revision 1
# baseline (speedup 1.0000x reference)
"""Trainium2 Bass kernel for batched tiny-graph GNN (B=32768, N=22 nodes).

Math per graph (A: [22,22], X: [22,16]):
  H1 = relu(A @ X @ W1 + b1)            -> restructured as relu(((A@X) @ W1) + b1)
  H2 = relu(A @ (H1@W2g) + H1@W2s + b2)
  g  = sum_n H2 ;  y = sigmoid(relu(g@Wf1+bf1) @ Wf2 + bf2)

Mapping: pure data parallelism over 8 cores (4096 graphs each).
On-chip layout: "tetrads" of 4 graphs at 32-aligned partition offsets so
per-graph matmuls get legal auto tile_position; activations kept
channel-major ([C, node-cols]) for weight-stationary f32r dense layers.
"""

import os
import sys

sys.path.insert(0, "/opt/trn_rl_repo")

import numpy as np

import concourse.bass as bass
import concourse.mybir as mybir
import concourse.tile as tile
from concourse.bass_utils import run_bass_kernel_spmd
from concourse.masks import make_identity


import bass_rust


def _patched_drain_and_barrier(self, tick_clock, wait_clock):
    """Walrus in this container cannot encode multi-wait Drain instructions;
    spread the tile-exit sem waits across single-wait NOPs instead."""
    from concourse.tile import ScopedClock

    probe = self.nc.sync.nop(hint="drain_wait_split")
    wait_clock.add_sem_waits(probe.ins, ScopedClock({None: tick_clock.global_clock}))
    si = probe.ins.sync_info
    waits = list(si.on_wait) if si is not None else []
    probe.ins.sync_info = bass_rust.SyncInfo(on_wait=waits[:1], on_update=[])
    for w in waits[1:]:
        n = self.nc.sync.nop(hint="drain_wait_split")
        n.ins.sync_info = bass_rust.SyncInfo(on_wait=[w], on_update=[])
    self.nc.sync.drain()
    self.nc.all_engine_barrier()
    assert self.sems is not None
    popped = self.nc._tile_sem_poison_stack.pop()
    assert popped is self._sem_poison
    self.nc.clear_and_free_semaphores(list(self.sems.allocated().values()))
    self.nc.all_engine_barrier()


tile.TileContext._drain_and_barrier = _patched_drain_and_barrier

F32 = mybir.dt.float32
F32R = mybir.dt.float32r

B_TOTAL = 32768
N_CORES = 8
B_CORE = B_TOTAL // N_CORES          # 4096
NN = 22                              # nodes per graph
FIN = 16
C1, C2, C3 = 128, 64, 32
WAVE = 16                            # graphs per wave (4 tetrads)
AFT = mybir.ActivationFunctionType

# MP2 operand bases differ (lhsT at 32g, rhs at 0); gamble on explicit
# tile_position. If CoreSim disagrees, set False for the safe (slower) path.
MP2_GAMBLE = False



_split_ctr = [0]


def _split_multi_waits(nc):
    """This container's walrus encodes at most one sem wait per instruction:
    hoist extra waits onto same-engine NOPs inserted just before."""
    for f in nc.m.functions:
        for bb in f.blocks:
            out = []
            for inst in bb.instructions:
                si = inst.sync_info
                if si is not None and len(si.on_wait) > 1:
                    waits = list(si.on_wait)
                    for w in waits[:-1]:
                        _split_ctr[0] += 1
                        n = mybir.InstNoOp(
                            name=f"waitsplit_{_split_ctr[0]}", ins=[], outs=[]
                        )
                        n.engine = inst.engine
                        n.sync_info = bass_rust.SyncInfo(on_wait=[w], on_update=[])
                        out.append(n)
                        nc.register_instruction(n, overwrite=True)
                    inst.sync_info = bass_rust.SyncInfo(
                        on_wait=[waits[-1]], on_update=list(si.on_update)
                    )
                out.append(inst)
            bb.instructions = out


def build_nc(b_core: int = B_CORE) -> bass.Bass:
    assert b_core % WAVE == 0
    n_waves = b_core // WAVE

    nc = bass.Bass()

    a_d = nc.declare_dram_parameter("a", [b_core, NN, NN], F32, isOutput=False)
    x_d = nc.declare_dram_parameter("x", [b_core, NN, FIN], F32, isOutput=False)
    w1_d = nc.declare_dram_parameter("w1", [FIN, C1], F32, isOutput=False)
    b1_d = nc.declare_dram_parameter("b1", [C1], F32, isOutput=False)
    w2g_d = nc.declare_dram_parameter("w2g", [C1, C2], F32, isOutput=False)
    w2s_d = nc.declare_dram_parameter("w2s", [C1, C2], F32, isOutput=False)
    b2_d = nc.declare_dram_parameter("b2", [C2], F32, isOutput=False)
    wf1_d = nc.declare_dram_parameter("wf1", [C2, C3], F32, isOutput=False)
    bf1_d = nc.declare_dram_parameter("bf1", [C3], F32, isOutput=False)
    wf2_d = nc.declare_dram_parameter("wf2", [C3, 1], F32, isOutput=False)
    bf2_d = nc.declare_dram_parameter("bf2", [1], F32, isOutput=False)
    y_d = nc.declare_dram_parameter("y", [b_core, 1], F32, isOutput=True)

    with tile.TileContext(nc) as tc:
        with (
            tc.tile_pool(name="const", bufs=1) as cpool,
            tc.tile_pool(name="ioa", bufs=3) as iopool,
            tc.tile_pool(name="iox", bufs=3) as ioxpool,
            tc.tile_pool(name="work", bufs=2) as wpool,
            tc.tile_pool(name="ps", bufs=1, space="PSUM") as pspool,
            tc.tile_pool(name="ps2", bufs=2, space="PSUM") as ps2pool,
        ):
            # ---- persistent constants ----
            ident = cpool.tile([128, 128], F32, tag="ident")
            make_identity(nc, ident[:, :])
            w1sb_t = cpool.tile([128, C1], F32, tag="w1")
            w1sb = w1sb_t[0:FIN]
            nc.sync.dma_start(out=w1sb[:, :], in_=w1_d[:, :])
            w2sb = cpool.tile([C1, 128], F32, tag="w2")
            nc.sync.dma_start(out=w2sb[:, 0:C2], in_=w2g_d[:, :])
            nc.sync.dma_start(out=w2sb[:, C2:128], in_=w2s_d[:, :])
            wf1sb_t = cpool.tile([128, C3], F32, tag="wf1")
            wf1sb = wf1sb_t[0:C2]
            nc.sync.dma_start(out=wf1sb[:, :], in_=wf1_d[:, :])
            wf2sb_t = cpool.tile([128, 1], F32, tag="wf2")
            wf2sb = wf2sb_t[0:C3]
            nc.sync.dma_start(out=wf2sb[:, :], in_=wf2_d[:, :])
            b1c = cpool.tile([C1, 1], F32, tag="b1")
            nc.sync.dma_start(out=b1c[:, :], in_=b1_d.rearrange("(c o) -> c o", o=1))
            # bias for [Zg | Zs] rows: 0 for Zg rows, b2 for Zs rows
            b2e = cpool.tile([128, 1], F32, tag="b2")
            nc.vector.memset(b2e[:, :], 0.0)
            nc.sync.dma_start(out=b2e[C2:128, :], in_=b2_d.rearrange("(c o) -> c o", o=1))
            bf1c_t = cpool.tile([128, 1], F32, tag="bf1")
            bf1c = bf1c_t[0:C3]
            nc.sync.dma_start(out=bf1c[:, :], in_=bf1_d.rearrange("(c o) -> c o", o=1))
            bf2c_t = cpool.tile([128, 1], F32, tag="bf2")
            bf2c = bf2c_t[0:1]
            nc.sync.dma_start(out=bf2c[:, :], in_=bf2_d.rearrange("(c o) -> c o", o=1))
            y_acc = cpool.tile([1, b_core], F32, tag="yacc")

            for w in range(n_waves):
                b0 = w * WAVE
                # ---- input DMAs ----
                # A: tetrad-stacked natural [128(32g+n), (t,m)]
                a_wave = iopool.tile([128, 4 * NN], F32, tag="a_wave")
                a_quad = a_d[b0 : b0 + WAVE].rearrange("(t g) n m -> g n t m", t=4)
                for g in range(4):
                    nc.sync.dma_start(
                        out=a_wave[32 * g : 32 * g + NN].rearrange(
                            "p (t m) -> p t m", t=4
                        ),
                        in_=a_quad[g],
                    )
                # X: horizontal [22(m), (j graphs, f)]
                x_wave_t = ioxpool.tile([128, WAVE * FIN], F32, tag="x_wave")
                x_wave = x_wave_t[0:NN]
                x_dst = x_wave.rearrange("p (j f) -> p j f", j=WAVE)
                x_src = x_d[b0 : b0 + WAVE].rearrange("j m f -> m j f")
                nc.sync.dma_start(out=x_dst, in_=x_src)

                # ---- A transposes: sAT[22(m), (t, 32g+n)] ----
                sAT_t = wpool.tile([128, 4 * 4 * NN], F32, tag="sAT")
                sAT = sAT_t[0:NN]
                for t in range(4):
                    pAT = pspool.tile([NN, 128], F32, tag="pat")
                    nc.tensor.transpose(
                        pAT[:, :],
                        a_wave[:, t * NN : (t + 1) * NN],
                        ident[:, :],
                    )
                    nc.scalar.copy(
                        out=sAT[:, t * 88 : (t + 1) * 88].rearrange(
                            "p (g n) -> p g n", g=4
                        ),
                        in_=pAT.rearrange("p (g q) -> p g q", g=4)[:, :, 0:NN],
                    )

                # ---- MP1 + transpose to channel-major AXT [16, 512] ----
                sAXT_t = wpool.tile([128, 512], F32, tag="sAXT")
                sAXT = sAXT_t[0:FIN]
                for t in range(4):
                    pAX = ps2pool.tile([128, FIN], F32, tag="pax")
                    nc.vector.memset(pAX[:, :], 0.0)
                    for g in range(4):
                        j = 4 * t + g
                        nc.tensor.matmul(
                            pAX[32 * g : 32 * g + NN, :],
                            lhsT=sAT[:, t * 88 + 22 * g : t * 88 + 22 * g + NN],
                            rhs=x_wave[:, j * FIN : (j + 1) * FIN],
                            tile_position=(0, 32 * g),
                        )
                    sAX = wpool.tile([128, FIN], F32, tag="sAX")
                    nc.vector.tensor_copy(sAX[:, :], pAX[:, :])
                    pAXT = pspool.tile([FIN, 128], F32, tag="paxt")
                    nc.tensor.transpose(
                        pAXT[:, :], sAX[:, :],
                        ident[:, :],
                    )
                    nc.scalar.copy(out=sAXT[:, t * 128 : (t + 1) * 128], in_=pAXT[:, :])

                # ---- dense1 (f32r): Z1T = (AX @ W1)^T ; relu+bias -> H1T ----
                pZ1T = pspool.tile([C1, 512], F32, tag="z1t")
                nc.tensor.matmul(
                    pZ1T[:, :], lhsT=w1sb[:, :], rhs=sAXT[:, :]
                )
                sH1T = wpool.tile([C1, 512], F32, tag="sH1T")
                nc.scalar.activation(
                    out=sH1T[:, :], in_=pZ1T[:, :], func=AFT.Relu, bias=b1c[:, :]
                )

                # ---- dense2 (f32r): [ZgT; ZsT] = (H1 @ [W2g|W2s])^T (+bias on Zs) ----
                pZT = pspool.tile([128, 512], F32, tag="zt")
                nc.tensor.matmul(
                    pZT[:, :], lhsT=w2sb[:, :], rhs=sH1T[:, :]
                )
                sZ = wpool.tile([128, 512], F32, tag="sZ")
                nc.vector.tensor_scalar(
                    out=sZ[:, :], in0=pZT[:, :], scalar1=b2e[:, :], scalar2=None,
                    op0=mybir.AluOpType.add,
                )

                # ---- MP2: pAZgT[64, (t, 32g+n)] = (A @ Zg)^T per graph ----
                pAZgT = pspool.tile([C2, 512], F32, tag="azgt")
                for t in range(4):
                    for g in range(4):
                        if MP2_GAMBLE:
                            nc.tensor.matmul(
                                pAZgT[:, t * 128 + 32 * g : t * 128 + 32 * g + NN],
                                lhsT=sZg[32 * g : 32 * g + NN, t * C2 : (t + 1) * C2],
                                rhs=sAT[:, t * 88 + 22 * g : t * 88 + 22 * g + NN],
                                tile_position=(0, 0),
                                skip_group_check=True,
                            )
                        else:
                            # safe path: re-transpose Zg slice to base 0 first
                            pZn = pspool.tile([NN, C2], F32, tag="pat")
                            nc.tensor.transpose(
                                pZn[:, :],
                                sZ[0:C2, t * 128 + 32 * g : t * 128 + 32 * g + NN],
                                ident[0:C2, 0:C2],
                            )
                            sZn = wpool.tile([NN, C2], F32, tag="sZn")
                            nc.vector.tensor_copy(sZn[:, :], pZn[:, :])
                            nc.tensor.matmul(
                                pAZgT[:, t * 128 + 32 * g : t * 128 + 32 * g + NN],
                                lhsT=sZn[:, :],
                                rhs=sAT[:, t * 88 + 22 * g : t * 88 + 22 * g + NN],
                            )

                # ---- H2T = relu(AZgT + ZsT(+b2)) on valid cols, compacted ----
                azg_v = pAZgT.rearrange("p (t q n) -> p t q n", t=4, q=4)[:, :, :, 0:NN]
                zs_v = sZ[C2:128, :].rearrange("p (t q n) -> p t q n", t=4, q=4)[:, :, :, 0:NN]
                sH2T_t = wpool.tile([128, WAVE * NN], F32, tag="sH2T")
                sH2T = sH2T_t[0:C2]
                h2_v = sH2T.rearrange("p (t q n) -> p t q n", t=4, q=4)
                nc.vector.tensor_tensor(
                    out=h2_v, in0=azg_v, in1=zs_v, op=mybir.AluOpType.add
                )
                nc.scalar.activation(
                    out=sH2T[:, :], in_=sH2T[:, :], func=AFT.Relu
                )

                # ---- pool over nodes + final MLP ----
                sG_t = wpool.tile([128, WAVE], F32, tag="sG")
                sG = sG_t[0:C2]
                nc.vector.reduce_sum(
                    out=sG[:, :],
                    in_=sH2T.rearrange("p (j n) -> p j n", j=WAVE),
                    axis=mybir.AxisListType.X,
                )
                pG1 = pspool.tile([C3, WAVE], F32, tag="pat")
                nc.tensor.matmul(pG1[:, :], lhsT=wf1sb[:, :], rhs=sG[:, :])
                sG1_t = wpool.tile([128, WAVE], F32, tag="sG1")
                sG1 = sG1_t[0:C3]
                nc.scalar.activation(
                    out=sG1[:, :], in_=pG1[:, :], func=AFT.Relu, bias=bf1c[:, :]
                )
                pY = pspool.tile([1, WAVE], F32, tag="paxt")
                nc.tensor.matmul(pY[:, :], lhsT=wf2sb[:, :], rhs=sG1[:, :])
                nc.scalar.activation(
                    out=y_acc[:, b0 : b0 + WAVE], in_=pY[:, :], func=AFT.Sigmoid,
                    bias=bf2c[:, :],
                )

            nc.sync.dma_start(
                out=y_d.rearrange("(o b) one -> o (b one)", o=1), in_=y_acc[:, :]
            )

    _split_multi_waits(nc)
    return nc


def kernel(**inputs) -> np.ndarray:
    x = np.asarray(inputs["x"], dtype=np.float32)
    a = np.asarray(inputs["a"], dtype=np.float32)
    weights = {
        k: np.asarray(inputs[k], dtype=np.float32)
        for k in ("w1", "b1", "w2g", "w2s", "b2", "wf1", "bf1", "wf2", "bf2")
    }

    nc = build_nc(B_CORE)
    in_maps = []
    for c in range(N_CORES):
        sl = slice(c * B_CORE, (c + 1) * B_CORE)
        m = {"x": x[sl], "a": a[sl]}
        m.update(weights)
        in_maps.append(m)

    res = run_bass_kernel_spmd(nc, in_maps, list(range(N_CORES)))
    outs = [res.results[c]["y"] for c in range(N_CORES)]
    return np.concatenate(outs, axis=0).astype(np.float32)


if __name__ == "__main__":
    rng = np.random.default_rng(0)
    demo = {
        "x": rng.standard_normal((B_TOTAL, NN, FIN), dtype=np.float32),
        "a": rng.random((B_TOTAL, NN, NN), dtype=np.float32),
        "w1": rng.standard_normal((FIN, C1), dtype=np.float32) * 0.1,
        "b1": np.zeros(C1, np.float32),
        "w2g": rng.standard_normal((C1, C2), dtype=np.float32) * 0.1,
        "w2s": rng.standard_normal((C1, C2), dtype=np.float32) * 0.1,
        "b2": np.zeros(C2, np.float32),
        "wf1": rng.standard_normal((C2, C3), dtype=np.float32) * 0.1,
        "bf1": np.zeros(C3, np.float32),
        "wf2": rng.standard_normal((C3, 1), dtype=np.float32) * 0.1,
        "bf2": np.zeros(1, np.float32),
    }
    y = kernel(**demo)
    print("out", y.shape, y.dtype, y[:4, 0])

